# revision 8
# baseline (speedup 1.0000x reference)
"""PointsRenderer (alpha compositing over K points/pixel) on 8 trn2 cores.

Sharding: data-parallel over batch B=8 -> 1 image per NeuronCore.

Per-fragment feature gather via InstDMAGatherAnt (SWDGE) on 4 parallel
descriptor queues: the [100000, 4] feature table is repacked on the host to
bf16 quad-row blocks (4 rows = 32 B payload) on the 256 B descriptor
stride, so an int16 block index reaches all 25000 blocks and each fragment
costs exactly one 32 B descriptor (half the HBM traffic of f32 quads).
A per-fragment 1-of-4 row select plus the compositing (weights,
front-to-back transmittance, K-sum) runs on DVE in bf16 (2x rate), with
the transmittance chain kept in f32; dists2 is streamed as fp16 d2/r^2
(host premultiplied, exact to ~5e-4).  SP streams inputs, ACT streams
outputs, everything double-buffered with explicit semaphores.

num_idxs is capped at 1024 by the gather ucode (Q7 scratch + read-pattern
limits - larger calls die with NRT_EXEC_UNIT_UNRECOVERABLE), so the gather
issue rate is the hard wall; desc-gen for different queues runs on
different Q7 cpu pairs and overlaps.
"""
from contextlib import ExitStack

import numpy as np

import concourse.bass as bass
import concourse.mybir as mybir
from concourse import bacc
from concourse.bass_utils import run_bass_kernel_spmd

B, H, W, K, P, C = 8, 512, 512, 8, 100000, 4
NPIX = H * W              # pixels per core (batch-parallel, 1 image/core)
NI = 1024                 # fragments per gather call (hard ucode max)
PIXC = NI // K            # 128 pixels per call -> 1 pixel per partition
CALLS = NPIX // PIXC      # 2048
GRP = 64                  # calls per pipeline group
NGRP = CALLS // GRP       # 32
NQ = 4                    # SWDGE descriptor queues (ucode max)
NB = P // 4               # 25000 quad-row blocks
DEPTH = 2
INV_R2 = 1e4              # 1/radius^2 for the staged problem (radius=0.01)

F32 = mybir.dt.float32
F16 = mybir.dt.float16
BF16 = mybir.dt.bfloat16
I16 = mybir.dt.int16
U8 = mybir.dt.uint8
OP = mybir.AluOpType


def raw_dma_gather(gpsimd, out_ap, in_ap, idxs_ap, num_idxs, elem_size,
                   stride_bytes_256, queue_num):
    """InstDMAGatherAnt with a 32 B payload on a 256 B HBM stride.

    bass.dma_gather asserts elem_size_bytes % 256 == 0 (a transpose-mode
    restriction); the non-transpose ucode path only quantizes the *stride*
    to 256 B, so construct the instruction directly.
    """
    _in_ap = gpsimd.lower_ap_dma(in_ap, for_custom_bir_dma=True)
    _idxs_ap = gpsimd.lower_ap(idxs_ap)
    _out_ap = gpsimd.lower_ap(out_ap)
    return gpsimd.add_instruction(mybir.InstDMAGatherAnt(
        name=gpsimd.bass.get_next_instruction_name(),
        ins=[*_in_ap, _idxs_ap, gpsimd.lower_val_access(gpsimd.to_reg(num_idxs))],
        outs=[_out_ap],
        transpose=False,
        num_idxs=num_idxs,
        elem_size=elem_size,
        stride_bytes_256=stride_bytes_256,
        gen_mode=0,
        single_packet=True,
        queue_num=queue_num,
    ))


def build(inv_r2: float = INV_R2, skip_compute: bool = False,
          skip_gather: bool = False):
    nc = bacc.Bacc(None, target_bir_lowering=False, debug=False,
                   num_swdge_queues=NQ)
    tq = nc.dram_tensor("tq", [NB, 128], BF16, kind="ExternalInput")
    ixh = nc.dram_tensor("ixh", [NGRP, 128, GRP * 64], I16, kind="ExternalInput")
    d2h = nc.dram_tensor("d2h", [NGRP, 128, GRP * K], F16, kind="ExternalInput")
    m8h = nc.dram_tensor("m8h", [NGRP, 128, GRP * K], U8, kind="ExternalInput")
    outh = nc.dram_tensor("outh", [NGRP, 128, GRP * C], F32, kind="ExternalOutput")

    with nc.Block() as block, ExitStack() as st:
        def sb(name, shape, dt):
            return st.enter_context(nc.sbuf_tensor(name, shape, dt))

        IX = [sb(f"ix{d}", [128, GRP * 64], I16) for d in range(DEPTH)]
        D2 = [sb(f"d2{d}", [128, GRP * K], F16) for d in range(DEPTH)]
        M8 = [sb(f"m8{d}", [128, GRP * K], U8) for d in range(DEPTH)]
        G = [sb(f"g{d}", [128, GRP * 128], BF16) for d in range(DEPTH)]
        OT = [sb(f"ot{d}", [128, GRP * C], F32) for d in range(DEPTH)]
        AL = sb("al", [128, GRP * K], F32)
        OM = sb("om", [128, GRP * K], F32)
        RT = sb("rt", [128, GRP], F32)
        CB = sb("cb", [128, GRP * K], BF16)
        MK = sb("mk", [128, GRP * K], BF16)
        SK = sb("sk", [128, GRP * K], BF16)
        TMP = sb("tmp", [128, GRP * K * C], BF16)
        ACC = sb("acc", [128, GRP * K * C], BF16)

        ld_sem = st.enter_context(nc.semaphore("ld"))
        g_sem = st.enter_context(nc.semaphore("gs"))
        cp_sem = st.enter_context(nc.semaphore("cp"))
        st_sem = st.enter_context(nc.semaphore("st"))

        @block.sync
        def _(sp: bass.BassEngine):
            for g in range(NGRP):
                d = g % DEPTH
                if g >= DEPTH:
                    # input tiles [d] are still being read by group g-DEPTH
                    sp.wait_ge(g_sem, GRP * 16 * (g - DEPTH + 1))
                    sp.wait_ge(cp_sem, g - DEPTH + 1)
                sp.dma_start(IX[d][:], ixh[g]).then_inc(ld_sem, 16)
                sp.dma_start(D2[d][:], d2h[g]).then_inc(ld_sem, 16)
                sp.dma_start(M8[d][:], m8h[g]).then_inc(ld_sem, 16)

        @block.scalar
        def _(act: bass.BassEngine):
            for g in range(NGRP):
                act.wait_ge(cp_sem, g + 1)
                act.dma_start(outh[g], OT[g % DEPTH][:]).then_inc(st_sem, 16)

        @block.gpsimd
        def _(gp: bass.BassGpSimd):
            if skip_gather:
                for g in range(NGRP):
                    gp.sem_inc(g_sem, GRP * 16)
                return
            for g in range(NGRP):
                d = g % DEPTH
                gp.wait_ge(ld_sem, 48 * g + 16)
                if g >= DEPTH:
                    gp.wait_ge(cp_sem, g - DEPTH + 1)  # G[d] consumed
                for c in range(GRP):
                    raw_dma_gather(
                        gp,
                        out_ap=G[d][:, c * 128:(c + 1) * 128]
                            .rearrange("p (j e) -> p j e", e=16),
                        in_ap=tq[:, 0:16],
                        idxs_ap=IX[d][:, 64 * c:64 * (c + 1)],
                        num_idxs=NI,
                        elem_size=16,
                        stride_bytes_256=1,
                        queue_num=c % NQ,
                    ).then_inc(g_sem, 16)

        @block.vector
        def _(v: bass.BassVectorEngine):
            if skip_compute:
                for g in range(NGRP):
                    v.wait_ge(ld_sem, 48 * (g + 1))
                    v.wait_ge(g_sem, GRP * 16 * (g + 1))
                    v.drain().then_inc(cp_sem, 1)
                return
            for g in range(NGRP):
                d = g % DEPTH
                v.wait_ge(ld_sem, 48 * (g + 1))
                v.wait_ge(g_sem, GRP * 16 * (g + 1))
                if g >= DEPTH:
                    v.wait_ge(st_sem, 16 * (g - DEPTH + 1))  # OT[d] flushed

                # a' = d2/r^2 (fp16, host-premultiplied): alpha = 1-a', om = a'
                v.tensor_scalar(AL[:], D2[d][:], -1.0, 1.0, OP.mult, OP.add)
                v.tensor_copy(OM[:], D2[d][:])
                v.drain()

                # contrib_k = alpha_k * prod_{j<k} om_j  (f32 chain -> bf16)
                cbv = CB[:].rearrange("p (c k) -> p c k", k=K)
                alv = AL[:].rearrange("p (c k) -> p c k", k=K)
                omv = OM[:].rearrange("p (c k) -> p c k", k=K)
                v.tensor_copy(cbv[:, :, 0], alv[:, :, 0])
                v.tensor_copy(RT[:], omv[:, :, 0])
                v.drain()
                for k in range(1, K):
                    v.tensor_mul(cbv[:, :, k], alv[:, :, k], RT[:])
                    if k < K - 1:
                        v.tensor_mul(RT[:], RT[:], omv[:, :, k])
                        v.drain()

                gv = G[d][:].rearrange("p (c k e) -> p c k e", k=K, e=16)
                accv = ACC[:].rearrange("p (c k ch) -> p c k ch", k=K, ch=C)
                tmpv = TMP[:].rearrange("p (c k ch) -> p c k ch", k=K, ch=C)
                skb = SK[:].rearrange("p (c k one) -> p c k one", k=K, one=1) \
                    .to_broadcast([128, GRP, K, C])
                v.drain()
                for kp in range(4):
                    # 1-of-4 row select mask straight off the u8 row ids
                    v.tensor_scalar(MK[:], M8[d][:], float(kp), None, OP.is_equal)
                    v.drain()
                    v.tensor_mul(SK[:], MK[:], CB[:])
                    v.drain()
                    if kp == 0:
                        v.tensor_mul(accv, gv[:, :, :, 0:C], skb)
                    else:
                        v.tensor_mul(tmpv, gv[:, :, :, 4 * kp:4 * kp + C], skb)
                        v.drain()
                        v.tensor_add(accv, accv, tmpv)
                    v.drain()

                # K-sum tree -> [128, GRP, C] (final add converts to f32)
                v.tensor_add(accv[:, :, 0:4, :], accv[:, :, 0:4, :],
                             accv[:, :, 4:8, :])
                v.drain()
                v.tensor_add(accv[:, :, 0:2, :], accv[:, :, 0:2, :],
                             accv[:, :, 2:4, :])
                v.drain()
                otv = OT[d][:].rearrange("p (c ch) -> p c ch", ch=C)
                v.tensor_add(otv, accv[:, :, 0, :], accv[:, :, 1, :])
                v.drain().then_inc(cp_sem, 1)

    nc.compile()
    return nc


def prep_core_inputs(idx_b: np.ndarray, d2_b: np.ndarray,
                     inv_r2: float = INV_R2):
    """Host-side marshalling for one core: block/row split of the indices,
    per-call K-major fragment streams, wrap-16 + queue-banded index layout,
    fp16 premultiplied d2/r^2."""
    r = np.ascontiguousarray(idx_b.reshape(NPIX, K)).astype(np.int32)
    q = (r >> 2).astype(np.int16)     # quad-row block id, < 25000
    m = (r & 3).astype(np.uint8)      # row within block

    # call stream position s = k*128 + p  ->  partition p, slot k
    S = q.reshape(CALLS, PIXC, K).transpose(0, 2, 1).reshape(CALLS, NI)
    # wrap-16: index s lives at [s % 16, s // 16]
    Wr = S.reshape(CALLS, NI // 16, 16).transpose(0, 2, 1)  # [CALLS, 16, 64]
    # [NGRP, GRP, 16, 64] -> replicate wrap-16 stream to all 8 partition groups
    Wg = Wr.reshape(NGRP, GRP, 16, 64)
    IX = np.ascontiguousarray(
        np.tile(Wg, (1, 1, 8, 1))            # [NGRP, GRP, 128, 64]
        .transpose(0, 2, 1, 3)               # [NGRP, 128, GRP, 64]
        .reshape(NGRP, 128, GRP * 64)).astype(np.int16)

    a = d2_b.reshape(NGRP, GRP, 128, K).astype(np.float64) * inv_r2
    D2H = np.ascontiguousarray(a.transpose(0, 2, 1, 3)
                               .reshape(NGRP, 128, GRP * K)).astype(np.float16)
    M8H = np.ascontiguousarray(m.reshape(NGRP, GRP, 128, K)
                               .transpose(0, 2, 1, 3)
                               .reshape(NGRP, 128, GRP * K))
    return IX, D2H, M8H


def prep_table(features: np.ndarray):
    import ml_dtypes
    TQ = np.zeros((NB, 128), ml_dtypes.bfloat16)
    TQ[:, 0:16] = np.ascontiguousarray(features, dtype=np.float32) \
        .reshape(NB, 16).astype(ml_dtypes.bfloat16)
    return TQ


def unpack_output(outh: np.ndarray):
    return (outh.reshape(NGRP, 128, GRP, C).transpose(0, 2, 1, 3)
            .reshape(H, W, C))


def kernel(idx, dists2, features, radius):
    idx = np.ascontiguousarray(idx)
    dists2 = np.ascontiguousarray(dists2, dtype=np.float32)
    r = float(np.asarray(radius).reshape(-1)[0])
    inv_r2 = 1.0 / (r * r)

    nc = build(inv_r2)
    TQ = prep_table(features)

    in_maps = []
    for b in range(B):
        IX, D2H, M8H = prep_core_inputs(idx[b], dists2[b], inv_r2)
        in_maps.append({"tq": TQ, "ixh": IX, "d2h": D2H, "m8h": M8H})

    res = run_bass_kernel_spmd(nc, in_maps, core_ids=list(range(B)))

    out = np.empty((B, H, W, C), dtype=np.float32)
    for b in range(B):
        out[b] = unpack_output(res.results[b]["outh"])
    return out


# revision 10
# speedup vs baseline: 1.1835x; 1.1835x over previous
"""PointsRenderer (alpha compositing over K points/pixel) on 8 trn2 cores.

Sharding: data-parallel over batch B=8 -> 1 image per NeuronCore.

Per-fragment feature gather via InstDMAGatherAnt (SWDGE) on 4 parallel
descriptor queues: the [100000, 4] feature table is repacked on the host to
bf16 quad-row blocks (4 rows = 32 B payload) on the 256 B descriptor
stride, so an int16 block index reaches all 25000 blocks and each fragment
costs exactly one 32 B descriptor (half the HBM traffic of f32 quads).
A per-fragment 1-of-4 row select plus the compositing (weights,
front-to-back transmittance, K-sum) runs on DVE in bf16 (2x rate), with
the transmittance chain kept in f32; dists2 is streamed as fp16 d2/r^2
(host premultiplied, exact to ~5e-4).  SP streams inputs, ACT streams
outputs, everything double-buffered with explicit semaphores.

num_idxs is capped at 1024 by the gather ucode (Q7 scratch + read-pattern
limits - larger calls die with NRT_EXEC_UNIT_UNRECOVERABLE), so the gather
issue rate is the hard wall; desc-gen for different queues runs on
different Q7 cpu pairs and overlaps.
"""
from contextlib import ExitStack

import numpy as np

import concourse.bass as bass
import concourse.mybir as mybir
from concourse import bacc
from concourse.bass_utils import run_bass_kernel_spmd

B, H, W, K, P, C = 8, 512, 512, 8, 100000, 4
NPIX = H * W              # pixels per core (batch-parallel, 1 image/core)
NI = 1024                 # fragments per gather call (hard ucode max)
PIXC = NI // K            # 128 pixels per call -> 1 pixel per partition
CALLS = NPIX // PIXC      # 2048
GRP = 128                 # calls per pipeline group
NGRP = CALLS // GRP       # 32
NQ = 4                    # SWDGE descriptor queues (ucode max)
NB = P // 4               # 25000 quad-row blocks
DEPTH = 2
INV_R2 = 1e4              # 1/radius^2 for the staged problem (radius=0.01)

F32 = mybir.dt.float32
F16 = mybir.dt.float16
BF16 = mybir.dt.bfloat16
I16 = mybir.dt.int16
U8 = mybir.dt.uint8
OP = mybir.AluOpType


def raw_dma_gather(gpsimd, out_ap, in_ap, idxs_ap, num_idxs, elem_size,
                   stride_bytes_256, queue_num, nreg=None):
    """InstDMAGatherAnt with a 32 B payload on a 256 B HBM stride.

    bass.dma_gather asserts elem_size_bytes % 256 == 0 (a transpose-mode
    restriction); the non-transpose ucode path only quantizes the *stride*
    to 256 B, so construct the instruction directly.
    """
    _in_ap = gpsimd.lower_ap_dma(in_ap, for_custom_bir_dma=True)
    _idxs_ap = gpsimd.lower_ap(idxs_ap)
    _out_ap = gpsimd.lower_ap(out_ap)
    return gpsimd.add_instruction(mybir.InstDMAGatherAnt(
        name=gpsimd.bass.get_next_instruction_name(),
        ins=[*_in_ap, _idxs_ap,
             nreg if nreg is not None
             else gpsimd.lower_val_access(gpsimd.to_reg(num_idxs))],
        outs=[_out_ap],
        transpose=False,
        num_idxs=num_idxs,
        elem_size=elem_size,
        stride_bytes_256=stride_bytes_256,
        gen_mode=0,
        single_packet=True,
        queue_num=queue_num,
    ))


def build(inv_r2: float = INV_R2, skip_compute: bool = False,
          skip_gather: bool = False):
    nc = bacc.Bacc(None, target_bir_lowering=False, debug=False,
                   num_swdge_queues=NQ)
    tq = nc.dram_tensor("tq", [NB, 128], BF16, kind="ExternalInput")
    ixh = nc.dram_tensor("ixh", [NGRP, 128, GRP * 64], I16, kind="ExternalInput")
    d2h = nc.dram_tensor("d2h", [NGRP, 128, GRP * K], F16, kind="ExternalInput")
    m8h = nc.dram_tensor("m8h", [NGRP, 128, GRP * K], U8, kind="ExternalInput")
    outh = nc.dram_tensor("outh", [NGRP, 128, GRP * C], F32, kind="ExternalOutput")

    with nc.Block() as block, ExitStack() as st:
        def sb(name, shape, dt):
            return st.enter_context(nc.sbuf_tensor(name, shape, dt))

        IX = [sb(f"ix{d}", [128, GRP * 64], I16) for d in range(DEPTH)]
        D2 = [sb(f"d2{d}", [128, GRP * K], F16) for d in range(DEPTH)]
        M8 = [sb(f"m8{d}", [128, GRP * K], U8) for d in range(DEPTH)]
        G = [sb(f"g{d}", [128, GRP * 128], BF16) for d in range(DEPTH)]
        OT = [sb(f"ot{d}", [128, GRP * C], F32) for d in range(DEPTH)]
        AL = sb("al", [128, GRP * K], F32)
        OM = sb("om", [128, GRP * K], F32)
        RT = sb("rt", [128, GRP], F32)
        CB = sb("cb", [128, GRP * K], BF16)
        MK = sb("mk", [128, GRP * K], BF16)
        SK = sb("sk", [128, GRP * K], BF16)
        TMP = sb("tmp", [128, GRP * K * C], BF16)
        ACC = sb("acc", [128, GRP * K * C], BF16)

        ld_sem = st.enter_context(nc.semaphore("ld"))
        g_sem = st.enter_context(nc.semaphore("gs"))
        cp_sem = st.enter_context(nc.semaphore("cp"))
        st_sem = st.enter_context(nc.semaphore("st"))

        @block.sync
        def _(sp: bass.BassEngine):
            for g in range(NGRP):
                d = g % DEPTH
                if g >= DEPTH:
                    # input tiles [d] are still being read by group g-DEPTH
                    sp.wait_ge(g_sem, GRP * 16 * (g - DEPTH + 1))
                    sp.wait_ge(cp_sem, g - DEPTH + 1)
                sp.dma_start(IX[d][:], ixh[g]).then_inc(ld_sem, 16)
                sp.dma_start(D2[d][:], d2h[g]).then_inc(ld_sem, 16)
                sp.dma_start(M8[d][:], m8h[g]).then_inc(ld_sem, 16)

        @block.scalar
        def _(act: bass.BassEngine):
            for g in range(NGRP):
                act.wait_ge(cp_sem, g + 1)
                act.dma_start(outh[g], OT[g % DEPTH][:]).then_inc(st_sem, 16)

        @block.gpsimd
        def _(gp: bass.BassGpSimd):
            if skip_gather:
                for g in range(NGRP):
                    gp.sem_inc(g_sem, GRP * 16)
                return
            nreg = gp.lower_val_access(gp.to_reg(NI))
            for g in range(NGRP):
                d = g % DEPTH
                gp.wait_ge(ld_sem, 48 * g + 16)
                if g >= DEPTH:
                    gp.wait_ge(cp_sem, g - DEPTH + 1)  # G[d] consumed
                for c in range(GRP):
                    raw_dma_gather(
                        gp,
                        out_ap=G[d][:, c * 128:(c + 1) * 128]
                            .rearrange("p (j e) -> p j e", e=16),
                        in_ap=tq[:, 0:16],
                        idxs_ap=IX[d][:, 64 * c:64 * (c + 1)],
                        num_idxs=NI,
                        elem_size=16,
                        stride_bytes_256=1,
                        queue_num=c % NQ,
                        nreg=nreg,
                    ).then_inc(g_sem, 16)

        @block.vector
        def _(v: bass.BassVectorEngine):
            if skip_compute:
                for g in range(NGRP):
                    v.wait_ge(ld_sem, 48 * (g + 1))
                    v.wait_ge(g_sem, GRP * 16 * (g + 1))
                    v.drain().then_inc(cp_sem, 1)
                return
            for g in range(NGRP):
                d = g % DEPTH
                v.wait_ge(ld_sem, 48 * (g + 1))
                v.wait_ge(g_sem, GRP * 16 * (g + 1))
                if g >= DEPTH:
                    v.wait_ge(st_sem, 16 * (g - DEPTH + 1))  # OT[d] flushed

                # a' = d2/r^2 (fp16, host-premultiplied): alpha = 1-a', om = a'
                v.tensor_scalar(AL[:], D2[d][:], -1.0, 1.0, OP.mult, OP.add)
                v.drain()

                # contrib_k = alpha_k * prod_{j<k} om_j  (f32 chain -> bf16)
                cbv = CB[:].rearrange("p (c k) -> p c k", k=K)
                alv = AL[:].rearrange("p (c k) -> p c k", k=K)
                omv = D2[d][:].rearrange("p (c k) -> p c k", k=K)
                v.tensor_copy(cbv[:, :, 0], alv[:, :, 0])
                v.tensor_copy(RT[:], omv[:, :, 0])
                v.drain()
                for k in range(1, K):
                    v.tensor_mul(cbv[:, :, k], alv[:, :, k], RT[:])
                    if k < K - 1:
                        v.tensor_mul(RT[:], RT[:], omv[:, :, k])
                        v.drain()

                gv = G[d][:].rearrange("p (c k e) -> p c k e", k=K, e=16)
                accv = ACC[:].rearrange("p (c k ch) -> p c k ch", k=K, ch=C)
                tmpv = TMP[:].rearrange("p (c k ch) -> p c k ch", k=K, ch=C)
                skb = SK[:].rearrange("p (c k one) -> p c k one", k=K, one=1) \
                    .to_broadcast([128, GRP, K, C])
                v.drain()
                for kp in range(4):
                    # 1-of-4 row select mask straight off the u8 row ids
                    v.tensor_scalar(MK[:], M8[d][:], float(kp), None, OP.is_equal)
                    v.drain()
                    v.tensor_mul(SK[:], MK[:], CB[:])
                    v.drain()
                    if kp == 0:
                        v.tensor_mul(accv, gv[:, :, :, 0:C], skb)
                    else:
                        v.tensor_mul(tmpv, gv[:, :, :, 4 * kp:4 * kp + C], skb)
                        v.drain()
                        v.tensor_add(accv, accv, tmpv)
                    v.drain()

                # K-sum tree -> [128, GRP, C] (final add converts to f32)
                v.tensor_add(accv[:, :, 0:4, :], accv[:, :, 0:4, :],
                             accv[:, :, 4:8, :])
                v.drain()
                v.tensor_add(accv[:, :, 0:2, :], accv[:, :, 0:2, :],
                             accv[:, :, 2:4, :])
                v.drain()
                otv = OT[d][:].rearrange("p (c ch) -> p c ch", ch=C)
                v.tensor_add(otv, accv[:, :, 0, :], accv[:, :, 1, :])
                v.drain().then_inc(cp_sem, 1)

    nc.compile()
    return nc


def prep_core_inputs(idx_b: np.ndarray, d2_b: np.ndarray,
                     inv_r2: float = INV_R2):
    """Host-side marshalling for one core: block/row split of the indices,
    per-call K-major fragment streams, wrap-16 + queue-banded index layout,
    fp16 premultiplied d2/r^2."""
    r = np.ascontiguousarray(idx_b.reshape(NPIX, K)).astype(np.int32)
    q = (r >> 2).astype(np.int16)     # quad-row block id, < 25000
    m = (r & 3).astype(np.uint8)      # row within block

    # call stream position s = k*128 + p  ->  partition p, slot k
    S = q.reshape(CALLS, PIXC, K).transpose(0, 2, 1).reshape(CALLS, NI)
    # wrap-16: index s lives at [s % 16, s // 16]
    Wr = S.reshape(CALLS, NI // 16, 16).transpose(0, 2, 1)  # [CALLS, 16, 64]
    # [NGRP, GRP, 16, 64] -> replicate wrap-16 stream to all 8 partition groups
    Wg = Wr.reshape(NGRP, GRP, 16, 64)
    IX = np.ascontiguousarray(
        np.tile(Wg, (1, 1, 8, 1))            # [NGRP, GRP, 128, 64]
        .transpose(0, 2, 1, 3)               # [NGRP, 128, GRP, 64]
        .reshape(NGRP, 128, GRP * 64)).astype(np.int16)

    a = d2_b.reshape(NGRP, GRP, 128, K).astype(np.float64) * inv_r2
    D2H = np.ascontiguousarray(a.transpose(0, 2, 1, 3)
                               .reshape(NGRP, 128, GRP * K)).astype(np.float16)
    M8H = np.ascontiguousarray(m.reshape(NGRP, GRP, 128, K)
                               .transpose(0, 2, 1, 3)
                               .reshape(NGRP, 128, GRP * K))
    return IX, D2H, M8H


def prep_table(features: np.ndarray):
    import ml_dtypes
    TQ = np.zeros((NB, 128), ml_dtypes.bfloat16)
    TQ[:, 0:16] = np.ascontiguousarray(features, dtype=np.float32) \
        .reshape(NB, 16).astype(ml_dtypes.bfloat16)
    return TQ


def unpack_output(outh: np.ndarray):
    return (outh.reshape(NGRP, 128, GRP, C).transpose(0, 2, 1, 3)
            .reshape(H, W, C))


def kernel(idx, dists2, features, radius):
    idx = np.ascontiguousarray(idx)
    dists2 = np.ascontiguousarray(dists2, dtype=np.float32)
    r = float(np.asarray(radius).reshape(-1)[0])
    inv_r2 = 1.0 / (r * r)

    nc = build(inv_r2)
    TQ = prep_table(features)

    in_maps = []
    for b in range(B):
        IX, D2H, M8H = prep_core_inputs(idx[b], dists2[b], inv_r2)
        in_maps.append({"tq": TQ, "ixh": IX, "d2h": D2H, "m8h": M8H})

    res = run_bass_kernel_spmd(nc, in_maps, core_ids=list(range(B)))

    out = np.empty((B, H, W, C), dtype=np.float32)
    for b in range(B):
        out[b] = unpack_output(res.results[b]["outh"])
    return out
